# revision 29
# baseline (speedup 1.0000x reference)
"""Trainium2 Bass kernel for nn_DL_R_sum_MRC (MIMO MRC rate-sum loss).

Math (per batch b, RB i, subcarrier j, user k), derived from reference:
  V[c,t]   : unnormalized complex precoder (from y_pred), per (b, i)
  N2[c]    = sum_t |V[c,t]|^2           (normalization folded into the logs)
  hv[r,c]  = sum_t H_k[t,r] * V[c,t]    (complex, unnormalized)
  HF = hv[:,k], G = hv[:,1-k]
  q_u  = sum_r |HF_r|^2
  u_u  = sum_r conj(HF_r) * G_r
  DEN  = N2_k * (sigma * q_u * N2_kb + |u_u|^2 * P_kb)
  NUM  = DEN + q_u^2 * P_k * N2_kb
  rate = (ln NUM - ln DEN) / ln 2
  loss = -sum rate / (B * 52)

Sharding: pure data-parallel over batch, 8 NeuronCores x 512 batch.
Each core reduces its rates to a [128, NCHUNK] partial-sum tile; host sums.

On-chip layouts (batch in partitions, 128 per chunk):
  H  (DMA):   (sc, t, r, e)    sc*128 + t*4 + r*2 + e     [P, 6656] per user
  H' (relay): (sc, r, te)      sc*128 + r*64 + 2t + e     -> (j,r) merge to jr
  V' (relay): (c, i, te)       c*832 + i*64 + 2t + e
  hv tiles:   (i, jr, c)       i*16 + jr*2 + c            per user, re/im
"""

import math
import sys

import numpy as np

sys.path.insert(0, "/opt/trn_rl_repo")

B_FULL = 4096
N_CORES = 8
NB = B_FULL // N_CORES  # 512 batch per core
P = 128                 # partitions per chunk
NCHUNK = NB // P        # 4 chunks
SIGMA = 0.1
NRB = 13
NSC = 52

H_FREE = NSC * 32 * 2 * 2   # 6656
Y_FREE = 64 * NRB * 2       # 1664
P_FREE = NRB * 2            # 26: (i, c)

_TRACE = {"on": False, "result": None}


def _ap(x, off, dims):
    """View of tile/dram AP `x` at element offset `off` with free dims [[step, count], ...]."""
    import concourse.bass as bass

    return bass.AP(tensor=x.tensor, offset=x.offset + off, ap=[list(x.ap[0])] + dims)


def _build(nc, repeat=1, parts="all"):
    from contextlib import ExitStack

    import concourse.tile as tile
    from concourse import mybir

    f32 = mybir.dt.float32
    bf16 = mybir.dt.bfloat16
    Alu = mybir.AluOpType
    Act = mybir.ActivationFunctionType
    Ax = mybir.AxisListType

    h1d = nc.dram_tensor("h1", [NB, H_FREE], f32, kind="ExternalInput").ap()
    h2d = nc.dram_tensor("h2", [NB, H_FREE], f32, kind="ExternalInput").ap()
    yd = nc.dram_tensor("yp", [NB, Y_FREE], f32, kind="ExternalInput").ap()
    pd = nc.dram_tensor("pm", [NB, P_FREE], f32, kind="ExternalInput").ap()
    outd = nc.dram_tensor("partial", [P, NCHUNK], f32, kind="ExternalOutput").ap()

    with tile.TileContext(nc) as tc, ExitStack() as ctx:
        hpool = ctx.enter_context(tc.tile_pool(name="hpool", bufs=2))
        hrpool = ctx.enter_context(tc.tile_pool(name="hrpool", bufs=2))
        vpool = ctx.enter_context(tc.tile_pool(name="vpool", bufs=2))
        vdpool = ctx.enter_context(tc.tile_pool(name="vdpool", bufs=2))
        wpool = ctx.enter_context(tc.tile_pool(name="wpool", bufs=2))
        hvpool = ctx.enter_context(tc.tile_pool(name="hvpool", bufs=2))
        epool = ctx.enter_context(tc.tile_pool(name="epool", bufs=4))
        persist = ctx.enter_context(tc.tile_pool(name="persist", bufs=1))

        racc = persist.tile([P, NCHUNK], f32)

        for ch in [c for _ in range(repeat) for c in range(NCHUNK)]:
            b0 = ch * P
            bsl = slice(b0, b0 + P)

            yt = vpool.tile([P, Y_FREE], f32, tag="yt")
            nc.sync.dma_start(out=yt, in_=yd[bsl, :])
            pt = vpool.tile([P, P_FREE], f32, tag="pt")
            nc.sync.dma_start(out=pt, in_=pd[bsl, :])

            # H' (sc, r, te) built from half-chunk staged loads:
            # hraw (sc, t, r, e) -> hp (sc, r, te), cast to bf16
            h1p = hrpool.tile([P, H_FREE], bf16, tag="h1p")
            h2p = hrpool.tile([P, H_FREE], bf16, tag="h2p")
            for hd, hp in ((h1d, h1p), (h2d, h2p)):
                for half in range(2):
                    off = half * 26 * 128
                    hraw = hpool.tile([P, H_FREE // 2], f32, tag="hraw",
                                      name=f"hraw{half}")
                    nc.sync.dma_start(out=hraw, in_=hd[bsl, off:off + 3328])
                    for r in range(2):
                        nc.scalar.copy(
                            _ap(hp, off + r * 64, [[128, 26], [2, 32], [1, 2]]),
                            _ap(hraw, r * 2, [[128, 26], [4, 32], [1, 2]]),
                        )

            # ---- relayouts (ScalarE) ----
            # V' (c, i, te) from yt (c, t, i, e); vboth = [vneg | vswap]
            # with vneg = (re, -im), vswap = (im, re), both (c, i, te)
            vp = vdpool.tile([P, Y_FREE], f32, tag="vp")
            for c in range(2):
                nc.scalar.copy(
                    _ap(vp, c * 832, [[2, 32], [64, 13], [1, 2]]),
                    _ap(yt, c * 832, [[26, 32], [2, 13], [1, 2]]),
                )
            vboth = vdpool.tile([P, 2 * Y_FREE], bf16, tag="vboth")
            nc.scalar.copy(_ap(vboth, 0, [[1, Y_FREE]]), vp)
            ng = _ap(vboth, 1, [[64, 26], [2, 32]])
            nc.scalar.mul(ng, ng, -1.0)
            nc.scalar.copy(_ap(vboth, Y_FREE, [[64, 26], [2, 32]]),
                           _ap(vp, 1, [[64, 26], [2, 32]]))
            nc.scalar.copy(_ap(vboth, Y_FREE + 1, [[64, 26], [2, 32]]),
                           _ap(vp, 0, [[64, 26], [2, 32]]))
            if parts == "dmaonly":
                nc.vector.tensor_copy(_ap(racc, ch, [[1, 1]]),
                                      _ap(h1p, 0, [[1, 1]]))
                continue

            # ---- N2[c, i] = sum_{te} v^2 ----
            ysq = wpool.tile([P, Y_FREE], f32, tag="ysq")
            nc.scalar.square(ysq, vp)
            n2 = epool.tile([P, 26], f32, tag="n2")  # (c, i)
            nc.vector.tensor_reduce(
                out=_ap(n2, 0, [[1, 26]]),
                in_=_ap(ysq, 0, [[64, 26], [1, 64]]),
                axis=Ax.X, op=Alu.add)

            # ---- products + reduce over te ----
            # hvall layout (part, k, i, jr, c): strides 416, 208, 16, 2, 1
            hvall = hvpool.tile([P, 832], f32, tag="hvall")
            nrb_eff = 0 if parts == "nocore" else NRB
            if parts in ("nocore", "prodonly"):
                nc.vector.memset(hvall, 1.0)
            for i in range(nrb_eff):
                for k, hp in ((0, h1p), (1, h2p)):
                    hview = _ap(hp, i * 512, [[64, 8], [0, 2], [1, 64]])
                    # pr2 (part, jr, c, te): both re/im parts in one tile
                    pr2 = wpool.tile([P, 2048], bf16, tag="pr2")
                    for part in range(2):
                        vview = _ap(vboth, part * Y_FREE + i * 64,
                                    [[0, 8], [832, 2], [1, 64]])
                        nc.vector.tensor_mul(
                            _ap(pr2, part * 1024, [[128, 8], [64, 2], [1, 64]]),
                            hview, vview)
                    if parts == "prodonly":
                        continue
                    # three bf16 half-add levels (2x mode), then f32 reduce
                    pl1 = wpool.tile([P, 1024], bf16, tag="pl1")
                    nc.vector.tensor_add(
                        _ap(pl1, 0, [[32, 32], [1, 32]]),
                        _ap(pr2, 0, [[64, 32], [1, 32]]),
                        _ap(pr2, 32, [[64, 32], [1, 32]]))
                    pl2 = wpool.tile([P, 512], bf16, tag="pl2")
                    nc.vector.tensor_add(
                        _ap(pl2, 0, [[16, 32], [1, 16]]),
                        _ap(pl1, 0, [[32, 32], [1, 16]]),
                        _ap(pl1, 16, [[32, 32], [1, 16]]))
                    pl3 = wpool.tile([P, 256], bf16, tag="pl3")
                    nc.vector.tensor_add(
                        _ap(pl3, 0, [[8, 32], [1, 8]]),
                        _ap(pl2, 0, [[16, 32], [1, 8]]),
                        _ap(pl2, 8, [[16, 32], [1, 8]]))
                    nc.vector.tensor_reduce(
                        out=_ap(hvall, k * 208 + i * 16,
                                [[416, 2], [2, 8], [1, 2]]),
                        in_=_ap(pl3, 0, [[8, 32], [1, 8]]),
                        axis=Ax.X, op=Alu.add)

            if parts == "prodonly":
                nc.vector.tensor_copy(_ap(racc, ch, [[1, 1]]),
                                      _ap(hvall, 0, [[1, 1]]))
                continue

            # ---- epilogue per user k ----
            rsum = epool.tile([P, 52], f32, tag="rsum")
            for k in range(2):
                kb = 1 - k
                ijr = [[16, 13], [2, 8]]  # (i, jr) views into hvall
                hfre = _ap(hvall, k * 208 + k, ijr)
                hfim = _ap(hvall, 416 + k * 208 + k, ijr)
                gre = _ap(hvall, k * 208 + kb, ijr)
                gim = _ap(hvall, 416 + k * 208 + kb, ijr)

                t1 = epool.tile([P, 104], f32, tag="t1")
                t2 = epool.tile([P, 104], f32, tag="t2")
                red_in = [[2, 52], [1, 2]]   # (ij, r) view of t1
                red_out = [[1, 52]]          # (ij)

                # q_u
                nc.vector.tensor_mul(t1, hfre, hfre)
                nc.vector.tensor_mul(t2, hfim, hfim)
                nc.vector.tensor_add(t1, t1, t2)
                qu = epool.tile([P, 52], f32, tag="qu")
                nc.vector.tensor_reduce(out=_ap(qu, 0, red_out),
                                        in_=_ap(t1, 0, red_in),
                                        axis=Ax.X, op=Alu.add)
                # u_re
                nc.vector.tensor_mul(t1, hfre, gre)
                nc.vector.tensor_mul(t2, hfim, gim)
                nc.vector.tensor_add(t1, t1, t2)
                ure = epool.tile([P, 52], f32, tag="ure")
                nc.vector.tensor_reduce(out=_ap(ure, 0, red_out),
                                        in_=_ap(t1, 0, red_in),
                                        axis=Ax.X, op=Alu.add)
                # u_im
                nc.vector.tensor_mul(t1, hfre, gim)
                nc.vector.tensor_mul(t2, hfim, gre)
                nc.vector.tensor_sub(t1, t1, t2)
                uim = epool.tile([P, 52], f32, tag="uim")
                nc.vector.tensor_reduce(out=_ap(uim, 0, red_out),
                                        in_=_ap(t1, 0, red_in),
                                        axis=Ax.X, op=Alu.add)
                # |u|^2
                uu2 = epool.tile([P, 52], f32, tag="uu2")
                nc.vector.tensor_mul(ure, ure, ure)
                nc.vector.tensor_mul(uim, uim, uim)
                nc.vector.tensor_add(uu2, ure, uim)

                bk = _ap(n2, 13 * k, [[1, 13], [0, 4]])
                bkb = _ap(n2, 13 * kb, [[1, 13], [0, 4]])
                pk = _ap(pt, k, [[2, 13], [0, 4]])
                pkb = _ap(pt, kb, [[2, 13], [0, 4]])

                den = epool.tile([P, 52], f32, tag="den")
                num = epool.tile([P, 52], f32, tag="num")
                # den = bk * (sigma*qu*bkb + uu2*pkb)
                nc.vector.scalar_tensor_tensor(
                    out=den, in0=qu, scalar=SIGMA, in1=bkb,
                    op0=Alu.mult, op1=Alu.mult)
                nc.vector.tensor_mul(num, uu2, pkb)  # num as scratch
                nc.vector.tensor_add(den, den, num)
                nc.vector.tensor_mul(den, den, bk)
                # num = den + qu^2 * pk * bkb
                nc.vector.tensor_mul(num, qu, qu)
                nc.vector.tensor_mul(num, num, pk)
                nc.vector.tensor_mul(num, num, bkb)
                nc.vector.tensor_add(num, num, den)

                nc.scalar.activation(den, den, Act.Ln)
                nc.scalar.activation(num, num, Act.Ln)
                if k == 0:
                    nc.vector.tensor_sub(rsum, num, den)
                else:
                    nc.vector.tensor_sub(num, num, den)
                    nc.vector.tensor_add(rsum, rsum, num)

            nc.vector.tensor_reduce(
                out=_ap(racc, ch, [[1, 1]]),
                in_=rsum, axis=Ax.X, op=Alu.add)

        nc.sync.dma_start(out=outd, in_=racc)

    return nc


def _make_program(repeat=1):
    from concourse import bacc

    nc = bacc.Bacc("TRN2", target_bir_lowering=False, debug=False,
                   num_devices=N_CORES)
    _build(nc, repeat=repeat)
    nc.compile()
    return nc


def kernel(H_dl_RB_1, H_dl_RB_2, P_marix, y_pred):
    from concourse.bass_utils import run_bass_kernel_spmd

    h1 = np.ascontiguousarray(np.asarray(H_dl_RB_1, dtype=np.float32)).reshape(B_FULL, H_FREE)
    h2 = np.ascontiguousarray(np.asarray(H_dl_RB_2, dtype=np.float32)).reshape(B_FULL, H_FREE)
    yp = np.ascontiguousarray(np.asarray(y_pred, dtype=np.float32)).reshape(B_FULL, Y_FREE)
    pm = np.ascontiguousarray(np.asarray(P_marix, dtype=np.float32)).reshape(B_FULL, P_FREE)

    nc = _make_program()
    in_maps = []
    for c in range(N_CORES):
        s = slice(c * NB, (c + 1) * NB)
        in_maps.append({"h1": h1[s], "h2": h2[s], "yp": yp[s], "pm": pm[s]})

    res = run_bass_kernel_spmd(nc, in_maps, list(range(N_CORES)),
                               trace=_TRACE["on"])
    _TRACE["result"] = res
    total = np.float64(0.0)
    for r in res.results:
        total += np.float64(r["partial"].astype(np.float64).sum())
    loss = -total / (math.log(2.0) * B_FULL * NSC)
    return np.float32(loss)
